# revision 1
# baseline (speedup 1.0000x reference)
"""Trainium2 Bass kernel for nn_Ansatz_44573170598544.

QMC ansatz: per-walker Jastrow + 2-qubit PQC backflow + two 8x8 complex
Slater log-determinants. Pure data parallel: 32768 walkers over 8 cores.

Host-precomputed transforms (validated vs reference in numpy):
  * PQC z = f^T Q f, f = kron of (cos,sin) of 4 half-angles;
    Q = V diag(lam) V^T -> z = sum_i lam_i (V^T f)_i^2: TensorE
    block-diag matmuls in feature-major layout via PE transposes.
  * Slater E[n,m] = exp(i r_n.k_m); kvec 0 is (0,0) -> column 0 all-ones
    -> LU step 0 is a row subtraction; then batched no-pivot LU on 7x7
    via stride-0 broadcast APs (8192 matrices/core).
  * logdet = 0.5*Ln(|det|^2) + i*atan2(Im,Re) per spin (matches
    log(sign)+logabs of slogdet; no branch-cut wrapping).

Layouts (per core, 4096 walkers):
  particle-major planes [128, 512]: particle m = part*512 + col.
  matrix (w,s) -> part p = (w*16+8*s)//512, chunk tc = ((w*16+8*s)%512)//8
    i.e. w = p*32 + tc//2, s = tc%2; its 8 particles are columns
    tc*8..tc*8+7 of partition p.  Walker w -> (p, tw=tc//2).
  Jastrow & output use the same walker mapping (p, tw).
"""
import sys
import numpy as np

sys.path.insert(0, "/opt/trn_rl_repo")

from concourse import bass, mybir, tile  # noqa: E402
from concourse.bass_utils import run_bass_kernel_spmd  # noqa: E402

F32 = mybir.dt.float32
BF16 = mybir.dt.bfloat16
ALU = mybir.AluOpType
ACT = mybir.ActivationFunctionType
AXL = mybir.AxisListType

NCORES = 8
B = 32768
BLOC = B // NCORES          # 4096 walkers/core
NW = BLOC // 128            # 32 walkers per partition
M = BLOC * 16               # 65536 particles/core
MC = M // 128               # 512 particle columns
MCH = MC // 2               # pqc/LU half: 256 cols = 32 mats/part
BSH = 32                    # matrices per partition per half
JCH = 8
JB = NW // JCH              # 8 walkers per jastrow chunk
PIH = float(np.pi / 2)
PI = float(np.pi)

C_JP = 0
C_SC = [6, 18]
C_BC = [10, 22]
C_BSN = [14, 26]
C_KX = 32
C_KY = 39
C_MHALF = 46
C_PIHC = 47
C_MONE = 48
C_TPI = 49
C_MTPI = 50
NCONS = 51
WQ = 272                    # per-q weights: V0T 128, V1T 128, RL0 8, RL1 8
NWV = 2 * WQ


def _host_precompute(inputs):
    def rz(t):
        e = np.exp(-0.5j * t)
        return np.diag([e, np.conj(e)])

    def ry(t):
        c, s = np.cos(0.5 * t), np.sin(0.5 * t)
        return np.array([[c, -s], [s, c]], complex)

    def euler(p):
        return rz(p[2]) @ ry(p[1]) @ rz(p[0])

    def entangler(t):
        I4 = np.eye(4, dtype=complex)
        dzz = np.array([1., -1., -1., 1.])
        XX = np.array([[0, 0, 0, 1], [0, 0, 1, 0], [0, 1, 0, 0],
                       [1, 0, 0, 0]], complex)
        YY = np.array([[0, 0, 0, -1], [0, 0, 1, 0], [0, 1, 0, 0],
                       [-1, 0, 0, 0]], complex)
        rzz = lambda a: np.diag(np.exp(-0.5j * a * dzz))
        rxx = np.cos(0.5 * t[1]) * I4 - 1j * np.sin(0.5 * t[1]) * XX
        ryy = np.cos(0.5 * t[2]) * I4 - 1j * np.sin(0.5 * t[2]) * YY
        return rzz(t[3]) @ ryy @ rxx @ rzz(t[0])

    cons = np.zeros((128, NCONS), np.float32)
    wv = np.zeros((128, NWV), np.float32)
    cons[:, C_JP:C_JP + 6] = np.asarray(inputs["jastrow_param"], np.float64)
    X = np.array([[0, 1], [1, 0]], complex)
    I2 = np.eye(2, dtype=complex)
    for q in range(2):
        sq = np.asarray(inputs["param_single_qubit"][q], np.float64)
        tq = np.asarray(inputs["param_two_qubit"][q], np.float64)
        enc = np.asarray(inputs["param_encoding"][q], np.float64)
        enc_b = np.asarray(inputs["param_encoding_bias"][q], np.float64)
        U1 = (np.kron(euler(sq[0, 2]), euler(sq[0, 3])) @ entangler(tq[0])
              @ np.kron(euler(sq[0, 0]), euler(sq[0, 1])))
        U2 = (np.kron(euler(sq[1, 2]), euler(sq[1, 3])) @ entangler(tq[1])
              @ np.kron(euler(sq[1, 0]), euler(sq[1, 1])))
        D1 = np.diag([1, -1j, -1j, -1]).astype(complex)
        M1 = U1 @ D1
        KS = [np.kron(I2, I2), np.kron(I2, -1j * X),
              np.kron(-1j * X, I2), np.kron(-1j * X, -1j * X)]
        T = np.zeros((4, 16), complex)
        for i1 in range(2):
            for i2 in range(2):
                e12 = np.zeros(4)
                e12[2 * i1 + i2] = 1.0
                base = M1 @ e12
                for i3 in range(2):
                    for i4 in range(2):
                        T[:, 8 * i1 + 4 * i2 + 2 * i3 + i4] = \
                            U2 @ KS[2 * i3 + i4] @ base
        Z0 = np.diag([1., 1., -1., -1.]).astype(complex)
        Z1 = np.diag([1., -1., 1., -1.]).astype(complex)
        for j, Z in enumerate((Z0, Z1)):
            Q = np.real(T.conj().T @ Z @ T)
            Q = 0.5 * (Q + Q.T)
            lam, V = np.linalg.eigh(Q)
            W = np.zeros((128, 128), np.float32)
            RL = np.zeros((128, 8), np.float32)
            idx = np.arange(16) * 8
            for c in range(8):
                W[np.ix_(idx + c, idx + c)] = V
                RL[idx + c, c] = lam
            wv[:, q * WQ + j * 128: q * WQ + (j + 1) * 128] = W
            wv[:, q * WQ + 256 + j * 8: q * WQ + 256 + (j + 1) * 8] = RL
        scale = np.array([enc[0, 0], enc[0, 1], enc[1, 0], enc[1, 1]]) * np.pi
        bias = 0.5 * np.array([enc_b[0, 0], enc_b[0, 1],
                               enc_b[1, 0], enc_b[1, 1]])
        cons[:, C_SC[q]:C_SC[q] + 4] = scale
        cons[:, C_BC[q]:C_BC[q] + 4] = bias + PIH
        cons[:, C_BSN[q]:C_BSN[q] + 4] = bias
    kv = np.asarray(inputs["kvecs"], np.float64)
    assert abs(kv[0]).max() < 1e-6, "kernel assumes kvecs[0] == 0"
    cons[:, C_KX:C_KX + 7] = kv[1:8, 0]
    cons[:, C_KY:C_KY + 7] = kv[1:8, 1]
    cons[:, C_MHALF] = -0.5
    cons[:, C_PIHC] = PIH
    cons[:, C_MONE] = -1.0
    cons[:, C_TPI] = 2 * PI
    cons[:, C_MTPI] = -2 * PI
    pc = np.asarray(inputs["param_classical"], np.float64)
    pcv = np.zeros((128, 4), np.float32)
    pcv[:, 0:2] = pc[0]
    pcv[:, 2:4] = pc[1]
    ident = np.eye(128, dtype=np.float32)
    return cons, wv, ident, pcv


def build(gpat=None, debug=False, loop_n=0, trace_sim=False):
    if gpat is None:
        gpat = ((-1, 0), (0, -1), (0, 1), (1, 0),
                (-1, -1), (-1, 1), (1, -1))
    nc = bass.Bass()
    x_d = nc.declare_dram_parameter("x", [BLOC, 32], F32, isOutput=False)
    NKC = NCONS + NWV + 128 + 4
    kc_d = nc.declare_dram_parameter("kc", [128, NKC], F32, isOutput=False)
    out_d = nc.declare_dram_parameter("out", [BLOC, 2], F32, isOutput=True)
    if debug:
        dbg_jas = nc.declare_dram_parameter("dbg_jas", [128, NW], F32,
                                            isOutput=True)
        dbg_z = nc.declare_dram_parameter("dbg_z", [128, 4, MC], F32,
                                          isOutput=True)
        dbg_E = nc.declare_dram_parameter("dbg_E", [128, 2, BSH, 8, 7], F32,
                                          isOutput=True)
        dbg_det = nc.declare_dram_parameter("dbg_det", [128, 4, BSH], F32,
                                            isOutput=True)

    xflat = x_d[:].rearrange("b c -> (b c)")

    _tc_holder = {}
    with tile.TileContext(nc, trace_sim=trace_sim) as tc:
        _tc_holder['tc'] = tc
        with (
            tc.tile_pool(name="const", bufs=1) as cpool,
            tc.tile_pool(name="pers", bufs=1) as pers,
            tc.tile_pool(name="jt", bufs=2) as jt,
            tc.tile_pool(name="pt", bufs=1) as pt,
            tc.tile_pool(name="gt", bufs=3) as gt,
            tc.tile_pool(name="et", bufs=2) as et,
            tc.tile_pool(name="lt", bufs=2) as lt,
            tc.tile_pool(name="st", bufs=2) as st,
            tc.tile_pool(name="ps_t", bufs=2, space="PSUM") as ps_t,
            tc.tile_pool(name="ps_g", bufs=1, space="PSUM") as ps_g,
            tc.tile_pool(name="ps_w", bufs=2, space="PSUM") as ps_w,
        ):
            kc = cpool.tile([128, NKC], F32, tag="kc")
            nc.sync.dma_start(kc[:], kc_d[:])
            cons = kc[:, 0:NCONS]
            wvt = kc[:, NCONS:NCONS + NWV]
            ident = kc[:, NCONS + NWV:NCONS + NWV + 128]
            pcv = kc[:, NCONS + NWV + 128:NCONS + NWV + 132]

            wvb = cpool.tile([128, NWV], BF16, tag="wvb")
            nc.scalar.copy(wvb[:], kc[:, NCONS:NCONS + NWV])
            identb = cpool.tile([128, 128], BF16, tag="identb")
            nc.scalar.copy(identb[:], ident)

            def cc(i):
                return cons[:, i:i + 1]

            # =============== Jastrow (walker (p,tw) mapping) ===========
            xin = pers.tile([128, NW, 32], F32, tag="xin")
            nc.sync.dma_start(
                xin[:], xflat.rearrange("(p tw c) -> p tw c",
                                        p=128, tw=NW, c=32))
            xall = xin[:].rearrange("p tw c -> p (tw c)").rearrange(
                "p (cc d) -> p cc d", d=2)
            # ScalarE warm-ups: observe each DMA queue once so no real ACT
            # instruction ever needs two semaphore waits (ISA limit is 1).
            wsc1 = cpool.tile([128, 1], F32, tag="wsc1")
            wsc2 = cpool.tile([128, 1], F32, tag="wsc2")
            nc.scalar.activation(wsc1[:], kc[:, 0:1], ACT.Copy)
            nc.scalar.activation(wsc2[:], xin[:, 0, 0:1], ACT.Copy)
            wps = ps_w.tile([8, 8], F32, tag="wps")
            nc.tensor.transpose(wps[:], ident[0:8, 0:8], ident[0:8, 0:8])
            import contextlib
            _lcm = tc.For_i(0, loop_n, 1) if loop_n else \
                contextlib.nullcontext()
            _lcm.__enter__()
            # =============== PQC backflow -> zplm[q][:,j,:] ============
            zplm = [pers.tile([128, 2, MC], F32, tag=f"zplm{q}",
                              name=f"zplm{q}") for q in range(2)]
            zpl = [[zplm[q][:, j] for j in range(2)] for q in range(2)]

            # jastrow emitted first: fills DVE/ACT gaps of the PE-heavy
            # PQC phase (no data deps besides xin)
            jas = pers.tile([128, NW], F32, tag="jas")
            xinb = pers.tile([128, NW, 32], BF16, tag="xinb")
            nc.scalar.copy(xinb[:], xin[:])
            for ch in range(JCH):
                XJc = xinb[:, ch * JB:(ch + 1) * JB, :]
                d = jt.tile([128, JB, 240], BF16, tag="jd")
                off = 0
                for o in range(1, 16):
                    Lg = 32 - 2 * o
                    nc.vector.tensor_sub(d[:, :, off:off + Lg],
                                         XJc[:, :, 0:Lg],
                                         XJc[:, :, 2 * o:32])
                    off += Lg
                nc.scalar.activation(d[:], d[:], ACT.Abs)
                nc.scalar.activation(d[:], d[:], ACT.Abs, bias=cc(C_MHALF))
                admin = jt.tile([128, JB, 240], BF16, tag="jadm")
                nc.vector.tensor_scalar(admin[:], d[:], -1.0, 0.5,
                                        ALU.mult, ALU.add)
                t1 = jt.tile([128, JB, 240], BF16, tag="jt1")
                nc.scalar.activation(t1[:], admin[:], ACT.Square)
                nc.scalar.activation(t1[:], t1[:], ACT.Square)
                nc.vector.scalar_tensor_tensor(d[:], t1[:], -2.0,
                                               admin[:], ALU.mult, ALU.add)
                nc.scalar.activation(d[:], d[:], ACT.Square)
                dv = d[:].rearrange("p b (q t) -> p b q t", t=2)
                s = jt.tile([128, JB, 120], BF16, tag="js")
                nc.vector.tensor_add(s[:], dv[:, :, :, 0], dv[:, :, :, 1])
                t = jt.tile([128, JB, 120], BF16, tag="jtt")
                nc.scalar.activation(t[:], s[:], ACT.Sqrt)
                p1 = jt.tile([128, JB, 120], BF16, tag="jpp1")
                nc.vector.tensor_scalar(p1[:], s[:], cc(C_JP + 4),
                                        cc(C_JP + 2), ALU.mult, ALU.add)
                nc.vector.scalar_tensor_tensor(p1[:], p1[:], 0.0, s[:],
                                               ALU.bypass, ALU.mult)
                nc.vector.tensor_scalar_add(p1[:], p1[:], cc(C_JP + 0))
                p2 = jt.tile([128, JB, 120], BF16, tag="jpp2")
                nc.vector.tensor_scalar(p2[:], s[:], cc(C_JP + 5),
                                        cc(C_JP + 3), ALU.mult, ALU.add)
                nc.vector.scalar_tensor_tensor(p2[:], p2[:], 0.0, s[:],
                                               ALU.bypass, ALU.mult)
                nc.vector.tensor_scalar_add(p2[:], p2[:], cc(C_JP + 1))
                nc.vector.scalar_tensor_tensor(p1[:], p1[:], 0.0, t[:],
                                               ALU.bypass, ALU.mult)
                nc.vector.scalar_tensor_tensor(p2[:], p2[:], 0.0, s[:],
                                               ALU.bypass, ALU.mult)
                nc.vector.tensor_add(p1[:], p1[:], p2[:])
                nc.vector.tensor_reduce(jas[:, ch * JB:(ch + 1) * JB],
                                        p1[:], axis=AXL.X, op=ALU.add)
            if debug:
                nc.sync.dma_start(dbg_jas[:], jas[:])

            outre = pers.tile([128, NW], F32, tag="outre")
            outim = pers.tile([128, NW], F32, tag="outim")
            for hf in range(2):
                c0 = hf * MCH
                csl = slice(c0, c0 + MCH)
                for q in range(2):
                    trig = pt.tile([128, 8, MCH], BF16, tag="trig")
                    for j in range(4):
                        coord = xall[:, c0:c0 + MCH, j % 2]
                        nc.scalar.activation(trig[:, 2 * j, :], coord,
                                             ACT.Sin, bias=cc(C_BC[q] + j),
                                             scale=cc(C_SC[q] + j))
                        nc.scalar.activation(trig[:, 2 * j + 1, :], coord,
                                             ACT.Sin, bias=cc(C_BSN[q] + j),
                                             scale=cc(C_SC[q] + j))
                    u = pt.tile([128, 2, 2, MCH], BF16, tag="u")
                    nc.vector.tensor_mul(
                        u[:],
                        trig[:, 0:2, :].unsqueeze(2).broadcast_to(
                            (128, 2, 2, MCH)),
                        trig[:, 2:4, :].unsqueeze(1).broadcast_to(
                            (128, 2, 2, MCH)))
                    v = pt.tile([128, 2, 2, MCH], BF16, tag="v")
                    nc.vector.tensor_mul(
                        v[:],
                        trig[:, 4:6, :].unsqueeze(2).broadcast_to(
                            (128, 2, 2, MCH)),
                        trig[:, 6:8, :].unsqueeze(1).broadcast_to(
                            (128, 2, 2, MCH)))
                    f = pt.tile([128, MCH // 8, 16, 8], BF16, tag="f")
                    fo = f[:].rearrange("p t (a b) c -> p a b t c", a=4)
                    nc.vector.tensor_mul(
                        fo,
                        u[:].rearrange("p a b (t c) -> p (a b) t c", c=8)
                            .unsqueeze(2).broadcast_to(
                                (128, 4, 4, MCH // 8, 8)),
                        v[:].rearrange("p a b (t c) -> p (a b) t c", c=8)
                            .unsqueeze(1).broadcast_to(
                                (128, 4, 4, MCH // 8, 8)))
                    for gl in range(8):
                        grp = hf * 8 + gl
                        ftp = ps_t.tile([128, 512], BF16, tag="ftp")
                        for gi in range(4):
                            ti = gl * 4 + gi
                            nc.tensor.transpose(
                                ftp[:, gi * 128:(gi + 1) * 128],
                                f[:, ti].rearrange("p a c -> p (a c)"),
                                identb[:])
                        ftr = gt.tile([128, 512], BF16, tag="ftr")
                        nc.vector.tensor_copy(ftr[:], ftp[:])
                        gp = ps_g.tile([128, 1024], F32, tag="gp")
                        ztp = ps_w.tile([128, 2, 4, 8], F32, tag="ztp")
                        for j in range(2):
                            nc.tensor.matmul(
                                gp[:, j * 512:(j + 1) * 512],
                                wvb[:, q * WQ + j * 128:
                                    q * WQ + (j + 1) * 128],
                                ftr[:])
                            gsq = gt.tile([128, 512], F32, tag="gsq")
                            nc.scalar.activation(
                                gsq[:], gp[:, j * 512:(j + 1) * 512],
                                ACT.Square)
                            for gi in range(4):
                                nc.tensor.matmul(
                                    ztp[:, j, gi, :],
                                    gsq[:, gi * 128:(gi + 1) * 128],
                                    wvt[:, q * WQ + 256 + j * 8:
                                        q * WQ + 256 + (j + 1) * 8])
                        nc.vector.tensor_copy(
                            zplm[q][:, :, grp * 32:(grp + 1) * 32],
                            ztp[:].rearrange("p j a b -> p j (a b)"))

                # xc planes for this half
                xrh = [et.tile([128, MCH], F32, tag=f"xrh{d}",
                               name=f"xrh{d}{hf}") for d in range(2)]
                xih = [et.tile([128, MCH], F32, tag=f"xih{d}",
                               name=f"xih{d}{hf}") for d in range(2)]
                for dd in range(2):
                    nc.vector.scalar_tensor_tensor(
                        xrh[dd][:], zpl[0][dd][:, csl], pcv[:, dd:dd + 1],
                        xall[:, csl, dd], ALU.mult, ALU.add)
                    nc.vector.tensor_scalar_mul(
                        xih[dd][:], zpl[1][dd][:, csl],
                        pcv[:, 2 + dd:3 + dd])

                # unit phasors with quadrant-reduced Sin args
                trg = {}
                for d2 in range(2):
                    msk = et.tile([128, MCH], F32, tag="emsk")
                    u2 = et.tile([128, MCH], F32, tag="eu")
                    nc.vector.tensor_scalar(msk[:], xrh[d2][:], 0.5, None,
                                            ALU.is_ge)
                    nc.vector.tensor_sub(u2[:], xrh[d2][:], msk[:])
                    for kind in ("s", "c"):
                        if kind == "c":
                            v2 = et.tile([128, MCH], F32, tag="ev")
                            m2 = et.tile([128, MCH], F32, tag="em2")
                            nc.vector.tensor_scalar(v2[:], u2[:], 0.25,
                                                    None, ALU.add)
                            nc.vector.tensor_scalar(m2[:], v2[:], 0.5,
                                                    None, ALU.is_ge)
                            nc.vector.tensor_sub(v2[:], v2[:], m2[:])
                            base = v2
                        else:
                            base = u2
                        sg = et.tile([128, MCH], F32, tag="esg")
                        nc.scalar.activation(sg[:], base[:], ACT.Sign)
                        ab = et.tile([128, MCH], F32, tag="eab")
                        nc.vector.tensor_mul(ab[:], base[:], sg[:])
                        hm = et.tile([128, MCH], F32, tag="ehm")
                        nc.vector.tensor_scalar(hm[:], ab[:], -1.0, 0.5,
                                                ALU.mult, ALU.add)
                        nc.vector.tensor_tensor(ab[:], ab[:], hm[:],
                                                ALU.min)
                        nc.vector.tensor_mul(ab[:], ab[:], sg[:])
                        o_ = et.tile([128, MCH], F32, tag=f"tr{kind}{d2}",
                                     name=f"tr{kind}{d2}{hf}")
                        nc.scalar.activation(o_[:], ab[:], ACT.Sin,
                                             scale=cc(C_TPI))
                        trg[(kind, d2)] = o_
                    for (sgn_, nm) in ((C_MTPI, "p"), (C_TPI, "m")):
                        o_ = et.tile([128, MCH], F32, tag=f"md{nm}{d2}",
                                     name=f"md{nm}{d2}{hf}")
                        nc.scalar.activation(o_[:], xih[d2][:], ACT.Exp,
                                             scale=cc(sgn_))
                        trg[(nm, d2)] = o_
                names = {}
                for d2 in range(2):
                    for pm, sgn_ in (("p", 1), ("m", -1)):
                        re = et.tile([128, MCH], F32, tag=f"fr{pm}{d2}",
                                     name=f"fr{pm}{d2}{hf}")
                        im = et.tile([128, MCH], F32, tag=f"fi{pm}{d2}",
                                     name=f"fi{pm}{d2}{hf}")
                        nc.vector.tensor_mul(re[:], trg[(pm, d2)][:],
                                             trg[("c", d2)][:])
                        nc.vector.tensor_mul(im[:], trg[(pm, d2)][:],
                                             trg[("s", d2)][:])
                        names[(d2, 1 if pm == "p" else -1)] = (re, im,
                                                               sgn_)
                cols = []
                for (gx, gy) in gpat:
                    if gx != 0 and gy == 0:
                        cols.append(names[(0, gx)])
                    elif gx == 0 and gy != 0:
                        cols.append(names[(1, gy)])
                    else:
                        xr_, xi_, sx = names[(0, gx)]
                        yr_, yi_, sy = names[(1, gy)]
                        pre = et.tile([128, MCH], F32, tag=f"pr{gx}{gy}",
                                      name=f"pr{gx}{gy}{hf}")
                        pim = et.tile([128, MCH], F32, tag=f"pi{gx}{gy}",
                                      name=f"pi{gx}{gy}{hf}")
                        t1_ = et.tile([128, MCH], F32, tag="ept1")
                        t2_ = et.tile([128, MCH], F32, tag="ept2")
                        nc.vector.tensor_mul(t1_[:], xr_[:], yr_[:])
                        nc.vector.tensor_mul(t2_[:], xi_[:], yi_[:])
                        nc.vector.tensor_tensor(
                            pre[:], t1_[:], t2_[:],
                            ALU.subtract if sx * sy > 0 else ALU.add)
                        nc.vector.tensor_mul(t1_[:], xi_[:], yr_[:])
                        nc.vector.tensor_mul(t2_[:], xr_[:], yi_[:])
                        if sx > 0 and sy > 0:
                            nc.vector.tensor_add(pim[:], t1_[:], t2_[:])
                            isn = 1
                        elif sx < 0 and sy < 0:
                            nc.vector.tensor_add(pim[:], t1_[:], t2_[:])
                            isn = -1
                        elif sx > 0:
                            nc.vector.tensor_sub(pim[:], t1_[:], t2_[:])
                            isn = 1
                        else:
                            nc.vector.tensor_sub(pim[:], t2_[:], t1_[:])
                            isn = 1
                        cols.append((pre, pim, isn))

                # A-build (fused step-0 of the LU: col0 of E is all-ones)
                Arr = et.tile([128, BSH, 7, 7], F32, tag="Ar")
                Aii = et.tile([128, BSH, 7, 7], F32, tag="Ai")
                for j, (re, im, isn) in enumerate(cols):
                    rev = re[:].rearrange("p (t n) -> p t n", n=8)
                    imv = im[:].rearrange("p (t n) -> p t n", n=8)
                    nc.vector.tensor_sub(
                        Arr[:, :, :, j], rev[:, :, 1:8],
                        rev[:, :, 0:1].broadcast_to((128, BSH, 7)))
                    if isn > 0:
                        nc.vector.tensor_sub(
                            Aii[:, :, :, j], imv[:, :, 1:8],
                            imv[:, :, 0:1].broadcast_to((128, BSH, 7)))
                    else:
                        nc.vector.tensor_sub(
                            Aii[:, :, :, j],
                            imv[:, :, 0:1].broadcast_to((128, BSH, 7)),
                            imv[:, :, 1:8])
                dre = lt.tile([128, BSH], F32, tag="dre")
                dim_ = lt.tile([128, BSH], F32, tag="dim")
                nc.vector.tensor_copy(dre[:], Arr[:, :, 0, 0])
                nc.vector.tensor_copy(dim_[:], Aii[:, :, 0, 0])
                for k in range(6):
                    r = 6 - k
                    pr = Arr[:, :, k, k]
                    pi = Aii[:, :, k, k]
                    t1 = st.tile([128, BSH], F32, tag="lt1")
                    t2 = st.tile([128, BSH], F32, tag="lt2")
                    nc.vector.tensor_mul(t1[:], pr, pr)
                    nc.vector.tensor_mul(t2[:], pi, pi)
                    nc.vector.tensor_add(t1[:], t1[:], t2[:])
                    rinv = st.tile([128, BSH], F32, tag="lrinv")
                    nc.vector.reciprocal(rinv[:], t1[:])
                    cr = st.tile([128, BSH], F32, tag="lcr")
                    ci = st.tile([128, BSH], F32, tag="lci")
                    nc.vector.tensor_mul(cr[:], pr, rinv[:])
                    nc.vector.tensor_mul(ci[:], pi, rinv[:])
                    colr = Arr[:, :, k + 1:7, k]
                    coli = Aii[:, :, k + 1:7, k]
                    crb = cr[:].unsqueeze(2).broadcast_to((128, BSH, r))
                    cib = ci[:].unsqueeze(2).broadcast_to((128, BSH, r))
                    u1 = st.tile([128, BSH, 6], F32, tag="lu1")
                    u2 = st.tile([128, BSH, 6], F32, tag="lu2")
                    lre = st.tile([128, BSH, 6], F32, tag="llre")
                    lim = st.tile([128, BSH, 6], F32, tag="llim")
                    u1v, u2v = u1[:, :, 0:r], u2[:, :, 0:r]
                    lrev, limv = lre[:, :, 0:r], lim[:, :, 0:r]
                    nc.vector.tensor_mul(u1v, colr, crb)
                    nc.vector.tensor_mul(u2v, coli, cib)
                    nc.vector.tensor_add(lrev, u1v, u2v)
                    nc.vector.tensor_mul(u1v, coli, crb)
                    nc.vector.tensor_mul(u2v, colr, cib)
                    nc.vector.tensor_sub(limv, u1v, u2v)
                    lreb = lrev.unsqueeze(3).broadcast_to((128, BSH, r, r))
                    limb = limv.unsqueeze(3).broadcast_to((128, BSH, r, r))
                    prow_r = Arr[:, :, k:k + 1, k + 1:7].broadcast_to(
                        (128, BSH, r, r))
                    prow_i = Aii[:, :, k:k + 1, k + 1:7].broadcast_to(
                        (128, BSH, r, r))
                    w1 = lt.tile([128, BSH, 6, 6], F32, tag="lw1")
                    w2 = lt.tile([128, BSH, 6, 6], F32, tag="lw2")
                    w1v, w2v = w1[:, :, 0:r, 0:r], w2[:, :, 0:r, 0:r]
                    nc.vector.tensor_mul(w1v, lreb, prow_r)
                    nc.vector.tensor_mul(w2v, limb, prow_i)
                    nc.vector.tensor_sub(w1v, w1v, w2v)
                    nc.vector.tensor_sub(Arr[:, :, k + 1:7, k + 1:7],
                                         Arr[:, :, k + 1:7, k + 1:7], w1v)
                    nc.vector.tensor_mul(w1v, lreb, prow_i)
                    nc.vector.tensor_mul(w2v, limb, prow_r)
                    nc.vector.tensor_add(w1v, w1v, w2v)
                    nc.vector.tensor_sub(Aii[:, :, k + 1:7, k + 1:7],
                                         Aii[:, :, k + 1:7, k + 1:7], w1v)
                    npr = Arr[:, :, k + 1, k + 1]
                    npi = Aii[:, :, k + 1, k + 1]
                    d1 = st.tile([128, BSH], F32, tag="ld1")
                    d2 = st.tile([128, BSH], F32, tag="ld2")
                    nre = st.tile([128, BSH], F32, tag="lnre")
                    nc.vector.tensor_mul(d1[:], dre[:], npr)
                    nc.vector.tensor_mul(d2[:], dim_[:], npi)
                    nc.vector.tensor_sub(nre[:], d1[:], d2[:])
                    nc.vector.tensor_mul(d1[:], dre[:], npi)
                    nc.vector.tensor_mul(d2[:], dim_[:], npr)
                    nc.vector.tensor_add(dim_[:], d1[:], d2[:])
                    nc.vector.tensor_copy(dre[:], nre[:])

                if debug:
                    nc.sync.dma_start(dbg_det[:, 2 * hf], dre[:])
                    nc.sync.dma_start(dbg_det[:, 2 * hf + 1], dim_[:])
                q1 = st.tile([128, BSH], F32, tag="oq1")
                q2 = st.tile([128, BSH], F32, tag="oq2")
                nc.vector.tensor_mul(q1[:], dre[:], dre[:])
                nc.vector.tensor_mul(q2[:], dim_[:], dim_[:])
                nc.vector.tensor_add(q1[:], q1[:], q2[:])
                labs = st.tile([128, BSH], F32, tag="olabs")
                nc.scalar.activation(labs[:], q1[:], ACT.Ln)
                rinv = st.tile([128, BSH], F32, tag="orinv")
                nc.vector.reciprocal(rinv[:], dre[:])
                nc.vector.tensor_mul(rinv[:], dim_[:], rinv[:])
                at = st.tile([128, BSH], F32, tag="oat")
                nc.scalar.activation(at[:], rinv[:], ACT.Arctan)
                mneg = st.tile([128, BSH], F32, tag="omneg")
                nc.vector.tensor_scalar(mneg[:], dre[:], 0.0, None,
                                        ALU.is_lt)
                sgn = st.tile([128, BSH], F32, tag="osgn")
                nc.scalar.activation(sgn[:], dim_[:], ACT.Sign)
                nc.vector.tensor_mul(mneg[:], mneg[:], sgn[:])
                nc.vector.scalar_tensor_tensor(at[:], mneg[:], PI, at[:],
                                               ALU.mult, ALU.add)
                labv = labs[:].rearrange("p (tw s) -> p tw s", s=2)
                argv = at[:].rearrange("p (tw s) -> p tw s", s=2)
                wsl = slice(hf * (NW // 2), (hf + 1) * (NW // 2))
                lsum = st.tile([128, NW // 2], F32, tag="olsum")
                nc.vector.tensor_add(lsum[:], labv[:, :, 0], labv[:, :, 1])
                nc.vector.scalar_tensor_tensor(
                    outre[:, wsl], lsum[:], 0.5, jas[:, wsl],
                    ALU.mult, ALU.add)
                nc.vector.tensor_add(outim[:, wsl], argv[:, :, 0],
                                     argv[:, :, 1])

            ov = out_d[:].rearrange("(p tw) r -> p tw r", p=128)
            nc.sync.dma_start(ov[:, :, 0:1], outre[:].unsqueeze(2))
            nc.sync.dma_start(ov[:, :, 1:2], outim[:].unsqueeze(2))
            _lcm.__exit__(None, None, None)
    _legalize_waits(nc)
    if trace_sim:
        return nc, _tc_holder['tc']
    return nc


def _legalize_waits(nc):
    """This walrus build allows only ONE sync wait per instruction;
    Tile emits several. Split extras onto EventSemaphore nops."""
    n = 0
    for fn in nc.m.functions:
        for b in fn.blocks:
            out = []
            for ins in b.instructions:
                si = ins.sync_info
                if si is not None and si.on_wait and len(si.on_wait) > 1:
                    waits = list(si.on_wait)
                    for i, w in enumerate(waits[:-1]):
                        out.append(mybir.InstEventSemaphore(
                            name=f"WSPLIT{n}-{ins.name}",
                            engine=ins.engine,
                            sync_info=mybir.SyncInfo(on_wait=[w],
                                                     on_update=[]),
                            ins=[], outs=[], debug=ins.debug))
                        n += 1
                    ins.sync_info = mybir.SyncInfo(
                        on_wait=[waits[-1]],
                        on_update=list(si.on_update or []))
                out.append(ins)
            b.instructions = out
    return n


_CACHE = {}


def make_in_maps(inputs):
    cons, wv, ident, pcv = _host_precompute(inputs)
    kc = np.concatenate([cons, wv, ident, pcv], axis=1)
    x = np.ascontiguousarray(np.asarray(inputs["x"], np.float32))
    return [{
        "x": x[c * BLOC:(c + 1) * BLOC], "kc": kc,
    } for c in range(NCORES)]


def kernel(**inputs):
    if "nc" not in _CACHE:
        _CACHE["nc"] = build()
    nc = _CACHE["nc"]
    in_maps = make_in_maps(inputs)
    res = run_bass_kernel_spmd(nc, in_maps, core_ids=list(range(NCORES)))
    outs = [res.results[c]["out"] for c in range(NCORES)]
    full = np.concatenate(outs, axis=0)
    return (full[:, 0] + 1j * full[:, 1]).astype(np.complex64)

